# revision 1
# baseline (speedup 1.0000x reference)
"""Distributed Trainium2 kernel for nn_Attention_14181982012033.

Math (reference): p = x @ W; per-head ph = split(p); q = ph/sqrt(d);
logits = q @ ph^T; w = softmax(logits); attn = w @ ph; out = merge(attn) @ W.
Shapes: x [4, 2048, 1024] f32, W [1024, 1024] f32, 16 heads, d = 64.

Sharding: 8 cores = 4 batches x 2 head-groups (8 heads each). Each core
computes the FULL 2048-query attention for its 8 heads plus the partial
output projection attn_g @ W[g*512:(g+1)*512, :]; the host sums the two
partials per batch (zero device collectives). Inputs per core: xT = x[b]^T
(bf16), Wg = W[:, g-cols], Wr = W[g-rows, :]. The head-group split halves
projection matmul work vs a query-split layout.

exp splits across two engines to break the ACT throughput ceiling (33.5M
exps/core at 1/cycle/lane = ~290us on ACT alone): ACT computes true exp for
head A of each pair; DVE computes Schraudolph bit-trick exp for head B --
int16(A*logit + B) written into a bf16 tile IS the bf16 encoding of
exp(logit*scale) to ~1.8% RMS, washed out by softmax normalization (B is
calibrated weight-unbiased). Kernel rel err 5.7e-3 vs the 2e-2 gate.

Pipeline hygiene (the difference between this and a 590us version):
key-tiles are processed in batches of 4 -- an 8-matmul gram burst (64-row
PE tiling mode, two overlappable T0/T8 pairs) then a 4-matmul AV burst
(full mode) -- because every tiling-mode change drains the PE array;
cutting the switch count 4x bought ~100us. All
attention PSUM tiles are single-bank ([128,512] gram / [65,512] AV) with a
6-buffer gram pool, giving the PE ~3 kt of lookahead so it never waits on
exp completion semaphores (micro-stalls trip the HAM clock throttle to 1.2
GHz). Softmax denominators ride a 65th ones column in p_pad through the AV
matmuls (PSUM row 64, evacuated to bf16 sums rows); normalization avoids
any DRAM round-trip: two K=1 masked bf16 matmuls broadcast the sums across
partitions in PSUM, the approx reciprocal runs immediately on the hot data,
and the in-place multiply of pair i is deferred into pair i+1's loop so the
strict-FIFO DVE queue never head-of-line blocks the exp stream.

The output-projection tail runs 8-matmul runs per query tile (both fc
halves accumulated before evacuation), evacuates on ACT while DVE finishes
the deferred normalization, and uses 6 evac buffers so the 8MB of output
DMA pipelines behind the matmul stream.
"""

import os
import sys
from contextlib import ExitStack

import numpy as np

for _p in ("/opt/trn_rl_repo", "/opt/pypackages"):
    if _p not in sys.path:
        sys.path.append(_p)

import ml_dtypes

import concourse.bass as bass
import concourse.bacc as bacc
import concourse.mybir as mybir
import concourse.tile as tile
from concourse.bass_utils import run_bass_kernel_spmd

B, S, H, NH, D = 4, 2048, 1024, 16, 64
HG = 512          # head-group width (8 heads x 64)
KT = H // 128     # 8 partition tiles along H
ST = S // 128     # 16 partition tiles along S
PAIRS = 4         # head pairs per core
DT = mybir.dt.bfloat16
F32 = mybir.dt.float32
I16 = mybir.dt.int16
SCALE = 1.0 / float(np.sqrt(D))

LOG2E = 1.4426950408889634
A16 = float(128.0 * LOG2E * SCALE)
B16 = float(128.0 * (127.0 - 0.057574))   # weighted-unbiased Schraudolph
AV_LAG = 4

_CACHE = {}


def _build():
    nc = bacc.Bacc()
    xT_d = nc.declare_dram_parameter("xT", [H, S], DT, isOutput=False)
    Wg_d = nc.declare_dram_parameter("Wg", [H, HG], DT, isOutput=False)
    Wr_d = nc.declare_dram_parameter("Wr", [HG, H], DT, isOutput=False)
    out_d = nc.declare_dram_parameter("out", [S, H], F32, isOutput=True)

    with ExitStack() as ctx:
        tc = ctx.enter_context(tile.TileContext(nc))
        res = ctx.enter_context(tc.tile_pool(name="res", bufs=1))
        work = ctx.enter_context(tc.tile_pool(name="work", bufs=2))
        epool = ctx.enter_context(tc.tile_pool(name="epool", bufs=8))
        evac = ctx.enter_context(tc.tile_pool(name="evac", bufs=6))
        psg = ctx.enter_context(tc.tile_pool(name="psg", bufs=6, space="PSUM"))
        psav = ctx.enter_context(tc.tile_pool(name="psav", bufs=1, space="PSUM"))
        dram = ctx.enter_context(tc.tile_pool(name="dram", bufs=2, space="DRAM"))

        # ---- load inputs
        xT, Wg = [], []
        for k in range(KT):
            t = res.tile([128, S], DT, tag=f"xT{k}", name=f"xT{k}")
            nc.sync.dma_start(out=t[:], in_=xT_d[k * 128:(k + 1) * 128, :])
            xT.append(t)
            w = res.tile([128, HG], DT, tag=f"Wg{k}", name=f"Wg{k}")
            nc.sync.dma_start(out=w[:], in_=Wg_d[k * 128:(k + 1) * 128, :])
            Wg.append(w)
        Wr = []
        for p in range(PAIRS):
            w = res.tile([128, H], DT, tag=f"Wr{p}", name=f"Wr{p}")
            nc.sync.dma_start(out=w[:], in_=Wr_d[p * 128:(p + 1) * 128, :])
            Wr.append(w)

        pT = [res.tile([128, S], DT, tag=f"pT{i}", name=f"pT{i}")
              for i in range(PAIRS)]
        # p natural, head-padded: 8 heads x 65 (64 values + ones column)
        p_pad = [res.tile([128, 8 * 65], DT, tag=f"pp{s}", name=f"pp{s}")
                 for s in range(ST)]
        for st in range(ST):
            v = p_pad[st][:].rearrange("p (h e) -> p h e", e=65)
            nc.vector.memset(v[:, :, 64:65], 1.0)
        attnT = [res.tile([128, S], DT, tag=f"at{i}", name=f"at{i}")
                 for i in range(PAIRS)]
        sums = [(res.tile([1, S], DT, tag=f"sma{i}", name=f"sma{i}"),
                 res.tile([1, S], DT, tag=f"smb{i}", name=f"smb{i}"))
                for i in range(PAIRS)]
        # partition-broadcast masks: maskA selects rows 0:64, maskB 64:128
        maskA = res.tile([1, 128], DT, tag="mka", name="mka")
        nc.vector.memset(maskA[:, 0:64], 1.0)
        nc.vector.memset(maskA[:, 64:128], 0.0)
        maskB = res.tile([1, 128], DT, tag="mkb", name="mkb")
        nc.vector.memset(maskB[:, 0:64], 0.0)
        nc.vector.memset(maskB[:, 64:128], 1.0)

        evac_tick = [0]

        def psum_evac(dst, src):
            # alternate ACT/DVE for PSUM evacuation copies
            if evac_tick[0] % 2 == 0:
                nc.scalar.copy(out=dst, in_=src)
            else:
                nc.vector.tensor_copy(out=dst, in_=src)
            evac_tick[0] += 1

        # ---- pT projection: pT[i] = (x @ Wg[:, i*128:+128])^T, [128, S]
        def proj_pT_group(i, sc):
            ps = psg.tile([128, 512], F32, tag="g", name="pjg")
            for k in range(KT):
                nc.tensor.matmul(
                    out=ps[:],
                    lhsT=Wg[k][:, i * 128:(i + 1) * 128],
                    rhs=xT[k][:, sc * 512:(sc + 1) * 512],
                    start=(k == 0),
                    stop=(k == KT - 1),
                )
            psum_evac(pT[i][:, sc * 512:(sc + 1) * 512], ps[:])

        # ---- p natural: p_pad[st] = (x @ Wg)[st rows], head-padded
        def p_pad_group(st):
            ps = psg.tile([128, 512], F32, tag="g", name="ppg")
            for k in range(KT):
                nc.tensor.matmul(
                    out=ps[:],
                    lhsT=xT[k][:, st * 128:(st + 1) * 128],
                    rhs=Wg[k][:, 0:512],
                    start=(k == 0),
                    stop=(k == KT - 1),
                )
            dst = p_pad[st][:].rearrange("p (h e) -> p h e", e=65)[:, :, 0:64]
            sv = ps[:].rearrange("p (h d) -> p h d", d=64)
            psum_evac(dst, sv)

        for sc in range(4):
            proj_pT_group(0, sc)
        p_pad_group(0)
        p_pad_group(1)

        # ---- attention: 512-query slots, 1-bank PSUM tiles, deep gram
        # lookahead (6 bufs) so the PE never waits on exp completion.
        deferred_norm = []

        def run_deferred():
            while deferred_norm:
                deferred_norm.pop(0)()

        for i in range(PAIRS):
            for qc in range(4):
                qb = qc * 512
                av0 = psav.tile([65, 512], F32, tag="av0", name="av0")
                av1 = psav.tile([65, 512], F32, tag="av1", name="av1")

                def do_av(kt, eA, eB, av0=av0, av1=av1, i=i):
                    st0, sp0 = (kt == 0), (kt == ST - 1)
                    vA = p_pad[kt][:, (2 * i) * 65:(2 * i + 1) * 65]
                    vB = p_pad[kt][:, (2 * i + 1) * 65:(2 * i + 2) * 65]
                    nc.tensor.matmul(out=av0[:], lhsT=vA,
                                     rhs=eA[:], start=st0, stop=sp0)
                    nc.tensor.matmul(out=av1[:], lhsT=vB,
                                     rhs=eB[:], start=st0, stop=sp0)

                pending = []
                for kt2 in range(0, ST, 4):
                    qs = slice(qb, qb + 512)
                    # 64-mode burst: 4 kt of T0/T8 gram pairs (8 MMs)
                    gtiles = []
                    for kt in range(kt2, kt2 + 4):
                        ks = slice(kt * 128, (kt + 1) * 128)
                        gA = psg.tile([128, 512], F32, tag="g", name="gA")
                        gB = psg.tile([128, 512], F32, tag="g", name="gB")
                        nc.tensor.matmul(out=gA[:], lhsT=pT[i][0:64, ks],
                                         rhs=pT[i][0:64, qs],
                                         start=True, stop=True)
                        nc.tensor.matmul(out=gB[:], lhsT=pT[i][64:128, ks],
                                         rhs=pT[i][64:128, qs],
                                         start=True, stop=True)
                        gtiles.append((kt, gA, gB))
                    for kt, gA, gB in gtiles:
                        eA = epool.tile([128, 512], DT, tag="eA", name="eA")
                        eB = epool.tile([128, 512], DT, tag="eB", name="eB")
                        for h, (gX, eX) in enumerate(((gA, eA), (gB, eB))):
                            if h == 0:
                                nc.scalar.activation(
                                    out=eX[:], in_=gX[:],
                                    func=mybir.ActivationFunctionType.Exp,
                                    scale=SCALE)
                            else:
                                nc.vector.tensor_scalar(
                                    out=eX[:].bitcast(I16), in0=gX[:],
                                    scalar1=A16, scalar2=B16,
                                    op0=mybir.AluOpType.mult,
                                    op1=mybir.AluOpType.add)
                        pending.append((kt, eA, eB))
                    # 128-mode burst: AV for the lagged kts (4 MMs)
                    while len(pending) > AV_LAG:
                        do_av(*pending.pop(0))

                    # stream projections (also 128-mode: no extra switch)
                    if i == 0 and qc == 0:
                        for st in range(kt2 + 2, min(kt2 + 6, ST)):
                            p_pad_group(st)
                    elif qc == 2 and i < PAIRS - 1:
                        proj_pT_group(i + 1, kt2 // 4)
                    if qc == 0 and kt2 == 8:
                        run_deferred()
                for args in pending:
                    do_av(*args)
                # evacuate numerators (DVE) and denominator rows (ACT)
                nc.vector.tensor_copy(out=attnT[i][0:64, qb:qb + 512],
                                      in_=av0[0:64, :])
                nc.vector.tensor_copy(out=attnT[i][64:128, qb:qb + 512],
                                      in_=av1[0:64, :])
                nc.scalar.copy(out=sums[i][0][:, qb:qb + 512],
                               in_=av0[64:65, :])
                nc.scalar.copy(out=sums[i][1][:, qb:qb + 512],
                               in_=av1[64:65, :])

            # ---- pair epilogue: broadcast the bf16 sums rows across
            # partitions with two K=1 masked matmuls per 512-chunk (PSUM
            # select-merge), reciprocal immediately (data is hot, no DMA
            # round-trip); only the TT multiply is deferred.
            rrep = work.tile([128, S], F32, tag="rrep", name="rrep", bufs=2)
            for q4 in range(4):
                qs = slice(q4 * 512, (q4 + 1) * 512)
                brd = psg.tile([128, 512], F32, tag="g", name="brd")
                nc.tensor.matmul(out=brd[:], lhsT=maskA[:],
                                 rhs=sums[i][0][:, qs],
                                 start=True, stop=False)
                nc.tensor.matmul(out=brd[:], lhsT=maskB[:],
                                 rhs=sums[i][1][:, qs],
                                 start=False, stop=True)
                nc.vector.reciprocal_approx_fast(out=rrep[:, qs], in_=brd[:])

            def norm(i=i, rrep=rrep):
                nc.vector.tensor_tensor(out=attnT[i][:], in0=attnT[i][:],
                                        in1=rrep[:], op=mybir.AluOpType.mult)
            deferred_norm.append(norm)

        run_deferred()

        # ---- output projection: out[q, :] = attnc_g @ Wr  (partial)
        # 8-MM runs per query tile (both fc halves), evacs on ACT only
        # (DVE finishes the deferred TT), deep evac buffering for the DMAs.
        for qt in range(S // 128):
            pss = []
            for fc in range(2):
                ps = psg.tile([128, 512], F32, tag="g", name="opg")
                for p in range(PAIRS):
                    nc.tensor.matmul(
                        out=ps[:],
                        lhsT=attnT[p][:, qt * 128:(qt + 1) * 128],
                        rhs=Wr[p][:, fc * 512:(fc + 1) * 512],
                        start=(p == 0),
                        stop=(p == PAIRS - 1),
                    )
                pss.append(ps)
            for fc, ps in enumerate(pss):
                ot = evac.tile([128, 512], F32, tag="ot", name="ot")
                nc.scalar.copy(out=ot[:], in_=ps[:])
                nc.sync.dma_start(
                    out=out_d[qt * 128:(qt + 1) * 128,
                              fc * 512:(fc + 1) * 512], in_=ot[:])
    nc.finalize()
    return nc


def _get_nc():
    if "nc" not in _CACHE:
        _CACHE["nc"] = _build()
    return _CACHE["nc"]


def _install_ntff_hook():
    """Register the axon NTFF profiling hook if this image's antenv lacks
    ``axon_hooks`` (test/profiling path only; grading never hits this)."""
    import types

    try:
        from antenv.axon_hooks import get_axon_ntff_profile_hook  # noqa: F401
        return
    except ImportError:
        pass
    import antenv

    mod = types.ModuleType("antenv.axon_hooks")
    state = {"hook": None}
    mod.set_axon_ntff_profile_hook = lambda h: state.__setitem__("hook", h)
    mod.get_axon_ntff_profile_hook = lambda: state["hook"]
    sys.modules["antenv.axon_hooks"] = mod
    antenv.axon_hooks = mod
    try:
        from trn_agent_boot.trn_boot import _ntff_profile_via_ctypes

        hook = _ntff_profile_via_ctypes("/opt/axon/libaxon_pjrt.so")
        mod.set_axon_ntff_profile_hook(hook)
    except Exception as e:  # degrade: tracing skipped, run still works
        print(f"ntff hook install failed: {e}", file=sys.stderr)


def _run(x, W, trace=False):
    if trace:
        _install_ntff_hook()
    nc = _get_nc()
    bf = ml_dtypes.bfloat16
    Wb = W.astype(bf)
    in_maps = []
    xTb = {}
    for c in range(8):
        b, g = divmod(c, 2)
        if b not in xTb:
            xTb[b] = np.ascontiguousarray(x[b].T).astype(bf)
        in_maps.append({
            "xT": xTb[b],
            "Wg": np.ascontiguousarray(Wb[:, g * HG:(g + 1) * HG]),
            "Wr": np.ascontiguousarray(Wb[g * HG:(g + 1) * HG, :]),
        })
    r = run_bass_kernel_spmd(nc, in_maps, core_ids=list(range(8)), trace=trace)
    y = np.empty((B, S, H), np.float32)
    for b in range(B):
        y[b] = r.results[2 * b]["out"]
        y[b] += r.results[2 * b + 1]["out"]
    _CACHE["last_result"] = r
    return y


def kernel(x, W):
    return _run(np.asarray(x, dtype=np.float32), np.asarray(W, dtype=np.float32),
                trace=bool(os.environ.get("BASS_KERNEL_TRACE")))

